# revision 35
# baseline (speedup 1.0000x reference)
"""Trainium2 Bass kernel for per-batch channel attention (CxAM-style).

Reference (per batch element b):
    q = (Wq @ x_b + bq)        # [64, T]
    k = (Wk @ x_b + bk)        # [64, T]
    v = (Wv @ x_b + bv)        # [512, T]
    R = q.T @ k                # [T, T]
    A = softmax(R, axis=-1)
    out_b = v @ A.T            # [512, T]

Sharding: pure data-parallel - batch B=8, one batch element per NeuronCore.

Design notes (all-bf16, PE-stream-minimal; fp8 was evaluated and rejected -
softmax weighting does not average away per-element quantization error, and
e5m2 exp / e4m3 V measured 5e-2 / 2.7e-2 rel err vs the 2e-2 budget):
  * All weights are pre-transposed/packed/cast to bf16 on the host, and x is
    pre-cast/packed to bf16 [128, tt, ci, 512] (contiguous per-tt DMA lines)
    - no PE transposes, no on-device casts, half the input DMA bytes.
  * A burst of dummy matmuls at t=0 keeps the PE HAM clock-gate warm through
    the ~9us input-DMA head (otherwise the first ~27us run at 1.2 GHz).
  * Scores per t-block of 512 are row-packed concurrent pairs (tile_position
    (0,0)/(64,0)) into a 2-bank PSUM tile, so ONE fused ACT exp over
    [128, 2, 512] amortizes the 352-cycle ACT instruction overhead.
  * Softmax denominator: bf16 pair-sums + q-level tree on DVE, with the four
    q-partials folded into a PSUM row by accumulating ones-matmuls spread
    over the block (removes 60 of 64 column-sum matmuls and shortens the
    serial chain at the block boundary). 1/denom is broadcast across
    partitions by GpSimd partition_broadcast (PE and DVE stay out of it).
  * Attention is software-pipelined at the pair level, one pair deep: the 8
    accumulating AV matmuls of pair g-1 issue right after the score pair of
    g, so the exp (ACT, ~1.1us) hides entirely under PE work and sc can be
    single-buffered.
  * PSUM budget: avs 4 banks + sc [128,2,512] 2 banks + proj 2 banks
    (dummies/qkproj/vproj/bv-broadcast/denominator row) = 8 banks.
"""

import os

os.environ.setdefault("MYCRO_LOCAL_CACHE", "1")

import numpy as np
import ml_dtypes

import concourse.bass as bass
import concourse.mybir as mybir
import concourse.tile as tile
from concourse import bacc
from concourse.bass_utils import run_bass_kernel_spmd

F32 = mybir.dt.float32
BF16 = mybir.dt.bfloat16
AF = mybir.ActivationFunctionType

B = 8
C = 512
T = 2048
CQ = 64
NCORES = 8

TB = 512            # t-block (free dim of main matmuls)
NTB = T // TB       # 4
NSC = T // 128      # 16 s-chunks
NPAIR = NSC // 2    # 8 row-packed score pairs per t-block
NCH = C // 128      # 4 contraction chunks
NCC = C // 128      # 4 output channel chunks
NDUMMY = 15         # HAM warmup matmuls (bridge the PE to the first x chunk)


def _build_program() -> bass.Bass:
    nc = bacc.Bacc("TRN2", target_bir_lowering=False, debug=False, num_devices=NCORES)

    # Host-prepared inputs (already transposed/packed/cast - see kernel()).
    x_d = nc.declare_dram_parameter("x", [128, NTB, NCH, TB], BF16, isOutput=False)
    wqkT_d = nc.declare_dram_parameter("wqkT", [128, NCH, 128], BF16, isOutput=False)
    wvT_d = nc.declare_dram_parameter("wvT", [128, NCH, C], BF16, isOutput=False)
    bqk_d = nc.declare_dram_parameter("bqk", [128, 1], F32, isOutput=False)
    bv_d = nc.declare_dram_parameter("bv", [1, C], BF16, isOutput=False)
    out_d = nc.declare_dram_parameter("out", [C, T], F32, isOutput=True)

    with tile.TileContext(nc) as tc:
        with (
            tc.tile_pool(name="const", bufs=1) as const,
            tc.tile_pool(name="weights", bufs=1) as wpool,
            tc.tile_pool(name="ps_proj", bufs=2, space="PSUM") as ps_proj,
            tc.tile_pool(name="ps_sc", bufs=1, space="PSUM") as ps_sc,
            tc.tile_pool(name="ps_av", bufs=1, space="PSUM") as ps_av,
            tc.tile_pool(name="et", bufs=4) as et_pool,
            tc.tile_pool(name="tree", bufs=1) as tree_pool,
            tc.tile_pool(name="small", bufs=2) as small,
            tc.tile_pool(name="rbp", bufs=2) as rb_pool,
            tc.tile_pool(name="outp", bufs=4) as outp,
        ):
            # ---- constants, warmup fodder
            junk = const.tile([128, TB], BF16)
            nc.gpsimd.memset(junk[:], 0.0)
            junk_out = const.tile([128, 16], BF16)
            ones_col = const.tile([128, 1], BF16)
            nc.gpsimd.memset(ones_col[:], 1.0)
            ones_row = const.tile([1, 128], BF16)
            nc.gpsimd.memset(ones_row[:], 1.0)

            # HAM warmup: keep the PE busy from t~0 so the clock gate opens
            # before the real matmuls start (junk data, one rotating bank).
            for i in range(NDUMMY):
                dmy = ps_proj.tile([128, TB], F32, tag="proj", name=f"dmy_{i}")
                nc.tensor.matmul(
                    dmy[:], junk[:, 0:128], junk[:], start=True, stop=True
                )
            # Preload the ACT exp table during the DMA head.
            nc.scalar.activation(junk_out[:], junk[:, 0:16], AF.Exp)

            # ---- raw inputs -> SBUF (qkproj-critical transfers first).
            # x is host-packed [128, tt, ci, TB] so each per-tt DMA moves
            # contiguous 4KB partition lines.
            x_s = wpool.tile([128, NTB, NCH, TB], BF16)
            for ci in range(NCH):
                nc.sync.dma_start(out=x_s[:, 0, ci], in_=x_d[:, 0, ci])
            wqkT = wpool.tile([128, NCH, 128], BF16)
            nc.sync.dma_start(out=wqkT[:], in_=wqkT_d[:])
            bqk = wpool.tile([128, 1], F32)
            nc.sync.dma_start(out=bqk[:], in_=bqk_d[:])
            wvT = wpool.tile([128, NCH, C], BF16)
            nc.sync.dma_start(out=wvT[:], in_=wvT_d[:])
            bv_row = wpool.tile([1, C], BF16)
            nc.sync.dma_start(out=bv_row[:], in_=bv_d[:])

            bv_bcast = wpool.tile([128, C], F32)
            qk = wpool.tile([128, T], BF16)   # rows 0:64 Q, 64:128 K
            kq = wpool.tile([128, T], BF16)   # rows 0:64 K, 64:128 Q
            vT = wpool.tile([128, NSC, C], BF16)

            def emit_proj(tt):
                tsl = slice(tt * TB, (tt + 1) * TB)
                if tt > 0:
                    # issued behind the previous tt's kq swaps on the Sync
                    # queue, so the early fabric bandwidth all goes to the
                    # transfers the projection head is actually waiting on
                    nc.sync.dma_start(out=x_s[:, tt], in_=x_d[:, tt])
                # packed Q/K projection: out rows 0:64 = Q, 64:128 = K
                ps = ps_proj.tile([128, TB], F32, tag="proj", name=f"qkp_{tt}")
                for ci in range(NCH):
                    nc.tensor.matmul(
                        ps[:],
                        wqkT[:, ci, :],
                        x_s[:, tt, ci, :],
                        start=(ci == 0),
                        stop=(ci == NCH - 1),
                    )
                nc.vector.tensor_scalar_add(qk[:, tsl], ps[:], bqk[:, 0:1])
                # swap-duplicate for row-packed score matmuls
                nc.sync.dma_start(out=kq[0:CQ, tsl], in_=qk[CQ:128, tsl])
                nc.sync.dma_start(out=kq[CQ:128, tsl], in_=qk[0:CQ, tsl])

                if tt == 0:
                    # bv broadcast [1, C] -> [128, C] (single bf16 matmul)
                    bvb = ps_proj.tile([128, C], F32, tag="proj", name="bvb")
                    nc.tensor.matmul(
                        bvb[:], ones_row[:], bv_row[:], start=True, stop=True
                    )
                    nc.vector.tensor_copy(bv_bcast[:], bvb[:])

                # V^T projection: vT[s, c] = x.T @ Wv.T + bv
                for jsub in range(NCH):
                    j = 4 * tt + jsub
                    psv = ps_proj.tile([128, C], F32, tag="proj", name=f"vp_{j}")
                    for ci in range(NCH):
                        nc.tensor.matmul(
                            psv[:],
                            x_s[:, tt, ci, jsub * 128:(jsub + 1) * 128],
                            wvT[:, ci, :],
                            start=(ci == 0),
                            stop=(ci == NCH - 1),
                        )
                    nc.vector.tensor_add(vT[:, j, :], psv[:], bv_bcast[:])

            # ---- attention: pair-level software pipeline, one pair deep.
            # Per pair slot the PE does: 2 concurrent score matmuls (~213ns)
            # + 8 accumulating AV matmuls of the previous pair (~1.7us); the
            # exp of the current pair (~1.1us on ACT) hides under that, so
            # sc can be single-buffered and the PE stream stays dense.
            state = {}   # tb -> block state
            avs = {}     # tb -> 4 PSUM accumulators

            def emit_scores_pair(tb, jj, st):
                tsl = slice(tb * TB, (tb + 1) * TB)
                j0, j1 = 2 * jj, 2 * jj + 1
                sc = ps_sc.tile([128, 2, TB], F32, tag="sc", name=f"sc_{tb}_{jj}")
                nc.tensor.matmul(
                    sc[:, 0, :],
                    kq[0:CQ, j0 * 128:(j0 + 1) * 128],
                    qk[0:CQ, tsl],
                    start=True,
                    stop=True,
                )
                nc.tensor.matmul(
                    sc[:, 1, :],
                    qk[CQ:128, j1 * 128:(j1 + 1) * 128],
                    kq[CQ:128, tsl],
                    start=True,
                    stop=True,
                    tile_position=(64, 0),
                )
                etp = et_pool.tile([128, 2, TB], BF16, tag="etp", name=f"etp_{tb}_{jj}")
                nc.scalar.activation(etp[:, :, :], sc[:, :, :], AF.Exp)
                st["etps"].append(etp)

            def emit_tree(tb, jj, st):
                # bf16 pairwise tree-sum toward the softmax denominator (DVE);
                # q-level partials fold straight into the dns PSUM accumulator
                # via ones-matmuls (4 per block), keeping the serial DVE chain
                # at the block boundary short. The reciprocal is kicked off as
                # soon as the last dns matmul lands.
                etp = st["etps"][jj]
                p = tree_pool.tile(
                    [128, TB], BF16, tag="tp", bufs=4, name=f"tp_{tb}_{jj}"
                )
                nc.vector.tensor_add(p[:], etp[:, 0, :], etp[:, 1, :])
                st["p"].append(p)
                if jj % 2 == 1:
                    q = tree_pool.tile(
                        [128, TB], BF16, tag="tq", bufs=3, name=f"tq_{tb}_{jj // 2}"
                    )
                    nc.vector.tensor_add(q[:], st["p"][jj - 1][:], st["p"][jj][:])
                    if jj == 1:
                        st["dns"] = ps_proj.tile(
                            [128, TB], F32, tag="proj", name=f"dns_{tb}"
                        )
                    nc.tensor.matmul(
                        st["dns"][0:1, :],
                        ones_col[:],
                        q[:],
                        start=(jj == 1),
                        stop=(jj == NPAIR - 1),
                    )
                if jj == NPAIR - 1:
                    rcol = small.tile([1, TB], F32, tag="rcol", name=f"rcol_{tb}")
                    nc.vector.reciprocal_approx_fast(rcol[:], st["dns"][0:1, :])
                    # broadcast 1/denom across partitions on the idle GpSimd
                    rb = rb_pool.tile([128, TB], F32, tag="rb", name=f"rb_{tb}")
                    nc.gpsimd.partition_broadcast(rb[:], rcol[:])
                    st["rb"] = rb

            def emit_consume(tb, jj, st):
                if jj == 0:
                    avs[tb] = [
                        ps_av.tile([128, TB], F32, tag=f"av{ck}", name=f"av{ck}_{tb}")
                        for ck in range(NCC)
                    ]
                etp = st["etps"][jj]
                last = jj == NPAIR - 1
                tsl = slice(tb * TB, (tb + 1) * TB)
                # ck-major order so av banks are first touched in the order
                # the previous block's normalize frees them; on the last pair
                # the reciprocal-broadcast matmul slots in after ck1 and the
                # per-ck normalize+store follows each finished accumulator.
                def emit_mul(cko):
                    ot = outp.tile([128, TB], F32, tag="ot", name=f"ot_{tb}_{cko}")
                    nc.vector.tensor_mul(ot[:], avs[tb][cko][:], st["rb"][:])
                    nc.sync.dma_start(
                        out=out_d[cko * 128:(cko + 1) * 128, tsl], in_=ot[:]
                    )

                for ck in range(NCC):
                    for idx in (0, 1):
                        j = 2 * jj + idx
                        nc.tensor.matmul(
                            avs[tb][ck][:],
                            vT[:, j, ck * 128:(ck + 1) * 128],
                            etp[:, idx, :],
                            start=(j == 0),
                            stop=(j == NSC - 1),
                        )
                    if last and ck >= 2:
                        emit_mul(ck - 2)
                if last:
                    emit_mul(2)
                    emit_mul(3)

            for tt in range(NTB):
                emit_proj(tt)

            pending = None  # (tb, jj)
            for tb in range(NTB):
                st = {"etps": [], "p": []}
                state[tb] = st
                for jj in range(NPAIR):
                    emit_scores_pair(tb, jj, st)
                    if pending is not None:
                        ptb, pjj = pending
                        emit_consume(ptb, pjj, state[ptb])
                        if pjj == NPAIR - 1:
                            del state[ptb]
                    emit_tree(tb, jj, st)
                    pending = (tb, jj)
            ptb, pjj = pending
            emit_consume(ptb, pjj, state[ptb])

    nc.compile()
    return nc


_PROGRAM = None


def _get_program() -> bass.Bass:
    global _PROGRAM
    if _PROGRAM is None:
        _PROGRAM = _build_program()
    return _PROGRAM


def _prep_inputs(inputs):
    x = np.ascontiguousarray(np.asarray(inputs["x"], dtype=np.float32))
    wq = np.asarray(inputs["Wq"], dtype=np.float32)
    bq = np.asarray(inputs["bq"], dtype=np.float32).reshape(CQ)
    wk = np.asarray(inputs["Wk"], dtype=np.float32)
    bk = np.asarray(inputs["bk"], dtype=np.float32).reshape(CQ)
    wv = np.asarray(inputs["Wv"], dtype=np.float32)
    bv = np.asarray(inputs["bv"], dtype=np.float32).reshape(C)

    bf = ml_dtypes.bfloat16
    # wqkT[p, ci, m]: m<64 -> Wq[m, ci*128+p], m>=64 -> Wk[m-64, ci*128+p]
    wqk = np.concatenate([wq, wk], axis=0)          # [128, C]
    wqkT = np.ascontiguousarray(
        wqk.T.reshape(NCH, 128, 128).transpose(1, 0, 2)
    ).astype(bf)                                     # [128, NCH, 128]
    # wvT[p, ci, c] = Wv[c, ci*128+p]
    wvT = np.ascontiguousarray(
        wv.T.reshape(NCH, 128, C).transpose(1, 0, 2)
    ).astype(bf)                                     # [128, NCH, C]
    bqk = np.concatenate([bq, bk]).reshape(128, 1).astype(np.float32)
    bv_row = np.ascontiguousarray(bv.reshape(1, C)).astype(bf)
    # x_bf[b][p, tt, ci, t'] = x[b, ci*128+p, tt*TB + t']
    x_bf = np.ascontiguousarray(
        x.reshape(B, NCH, 128, NTB, TB).transpose(0, 2, 3, 1, 4)
    ).astype(bf)                                     # [B, 128, NTB, NCH, TB]

    return [
        {
            "x": np.ascontiguousarray(x_bf[b]),
            "wqkT": wqkT,
            "wvT": wvT,
            "bqk": bqk,
            "bv": bv_row,
        }
        for b in range(NCORES)
    ]


def kernel(**inputs: np.ndarray) -> np.ndarray:
    nc = _get_program()
    in_maps = _prep_inputs(inputs)
    res = run_bass_kernel_spmd(nc, in_maps, list(range(NCORES)))
    out = np.stack([res.results[b]["out"] for b in range(NCORES)], axis=0)
    return out.astype(np.float32)


if __name__ == "__main__":
    import reference

    inputs = {k: np.asarray(v) for k, v in reference.setup_inputs().items()}
    expected = np.asarray(reference.reference(**inputs))
    actual = kernel(**inputs)
    rel = np.linalg.norm(actual - expected) / np.linalg.norm(expected)
    print("Relative error:", rel)


# revision 37
# speedup vs baseline: 1.0242x; 1.0242x over previous
"""Trainium2 Bass kernel for per-batch channel attention (CxAM-style).

Reference (per batch element b):
    q = (Wq @ x_b + bq)        # [64, T]
    k = (Wk @ x_b + bk)        # [64, T]
    v = (Wv @ x_b + bv)        # [512, T]
    R = q.T @ k                # [T, T]
    A = softmax(R, axis=-1)
    out_b = v @ A.T            # [512, T]

Sharding: pure data-parallel - batch B=8, one batch element per NeuronCore.

Design notes (all-bf16, PE-stream-minimal; fp8 was evaluated and rejected -
softmax weighting does not average away per-element quantization error, and
e5m2 exp / e4m3 V measured 5e-2 / 2.7e-2 rel err vs the 2e-2 budget):
  * All weights are pre-transposed/packed/cast to bf16 on the host, and x is
    pre-cast/packed to bf16 [128, tt, ci, 512] (contiguous per-tt DMA lines)
    - no PE transposes, no on-device casts, half the input DMA bytes.
  * A burst of dummy matmuls at t=0 keeps the PE HAM clock-gate warm through
    the ~9us input-DMA head (otherwise the first ~27us run at 1.2 GHz).
  * Scores per t-block of 512 are row-packed concurrent pairs (tile_position
    (0,0)/(64,0)) into a 2-bank PSUM tile, so ONE fused ACT exp over
    [128, 2, 512] amortizes the 352-cycle ACT instruction overhead.
  * Softmax denominator: bf16 pair-sums + q-level tree on DVE, with the four
    q-partials folded into a PSUM row by accumulating ones-matmuls spread
    over the block (removes 60 of 64 column-sum matmuls and shortens the
    serial chain at the block boundary). 1/denom is broadcast across
    partitions by GpSimd partition_broadcast (PE and DVE stay out of it).
  * Attention is software-pipelined at the pair level, one pair deep: the 8
    accumulating AV matmuls of pair g-1 issue right after the score pair of
    g, so the exp (ACT, ~1.1us) hides entirely under PE work and sc can be
    single-buffered.
  * PSUM budget: avs 4 banks + sc [128,2,512] 2 banks + proj 2 banks
    (dummies/qkproj/vproj/bv-broadcast/denominator row) = 8 banks.
"""

import os

os.environ.setdefault("MYCRO_LOCAL_CACHE", "1")

import numpy as np
import ml_dtypes

import concourse.bass as bass
import concourse.mybir as mybir
import concourse.tile as tile
from concourse import bacc
from concourse.bass_utils import run_bass_kernel_spmd

F32 = mybir.dt.float32
BF16 = mybir.dt.bfloat16
AF = mybir.ActivationFunctionType

B = 8
C = 512
T = 2048
CQ = 64
NCORES = 8

TB = 512            # t-block (free dim of main matmuls)
NTB = T // TB       # 4
NSC = T // 128      # 16 s-chunks
NPAIR = NSC // 2    # 8 row-packed score pairs per t-block
NCH = C // 128      # 4 contraction chunks
NCC = C // 128      # 4 output channel chunks
NDUMMY = 17         # HAM warmup matmuls (bridge the PE to the first x chunk)


def _build_program() -> bass.Bass:
    nc = bacc.Bacc("TRN2", target_bir_lowering=False, debug=False, num_devices=NCORES)

    # Host-prepared inputs (already transposed/packed/cast - see kernel()).
    x_d = nc.declare_dram_parameter("x", [128, NTB, NCH, TB], BF16, isOutput=False)
    wqkT_d = nc.declare_dram_parameter("wqkT", [128, NCH, 128], BF16, isOutput=False)
    wvT_d = nc.declare_dram_parameter("wvT", [128, NCH, C], BF16, isOutput=False)
    bqk_d = nc.declare_dram_parameter("bqk", [128, 1], F32, isOutput=False)
    bv_d = nc.declare_dram_parameter("bv", [1, C], BF16, isOutput=False)
    out_d = nc.declare_dram_parameter("out", [C, T], F32, isOutput=True)

    with tile.TileContext(nc) as tc:
        with (
            tc.tile_pool(name="const", bufs=1) as const,
            tc.tile_pool(name="weights", bufs=1) as wpool,
            tc.tile_pool(name="ps_proj", bufs=2, space="PSUM") as ps_proj,
            tc.tile_pool(name="ps_sc", bufs=1, space="PSUM") as ps_sc,
            tc.tile_pool(name="ps_av", bufs=1, space="PSUM") as ps_av,
            tc.tile_pool(name="et", bufs=4) as et_pool,
            tc.tile_pool(name="tree", bufs=1) as tree_pool,
            tc.tile_pool(name="small", bufs=2) as small,
            tc.tile_pool(name="rbp", bufs=2) as rb_pool,
            tc.tile_pool(name="outp", bufs=4) as outp,
        ):
            # ---- constants, warmup fodder
            junk = const.tile([128, TB], BF16)
            nc.gpsimd.memset(junk[:], 0.0)
            junk_out = const.tile([128, 16], BF16)
            ones_col = const.tile([128, 1], BF16)
            nc.gpsimd.memset(ones_col[:], 1.0)
            ones_row = const.tile([1, 128], BF16)
            nc.gpsimd.memset(ones_row[:], 1.0)

            # HAM warmup: keep the PE busy from t~0 so the clock gate opens
            # before the real matmuls start (junk data, one rotating bank).
            for i in range(NDUMMY):
                dmy = ps_proj.tile([128, TB], F32, tag="proj", name=f"dmy_{i}")
                nc.tensor.matmul(
                    dmy[:], junk[:, 0:128], junk[:], start=True, stop=True
                )
            # Preload the ACT exp table during the DMA head.
            nc.scalar.activation(junk_out[:], junk[:, 0:16], AF.Exp)

            # ---- raw inputs -> SBUF (qkproj-critical transfers first).
            # x is host-packed [128, tt, ci, TB] so each per-tt DMA moves
            # contiguous 4KB partition lines.
            x_s = wpool.tile([128, NTB, NCH, TB], BF16)
            for ci in range(NCH):
                nc.sync.dma_start(out=x_s[:, 0, ci], in_=x_d[:, 0, ci])
            wqkT = wpool.tile([128, NCH, 128], BF16)
            nc.sync.dma_start(out=wqkT[:], in_=wqkT_d[:])
            bqk = wpool.tile([128, 1], F32)
            nc.sync.dma_start(out=bqk[:], in_=bqk_d[:])
            wvT = wpool.tile([128, NCH, C], BF16)
            nc.sync.dma_start(out=wvT[:], in_=wvT_d[:])
            bv_row = wpool.tile([1, C], BF16)
            nc.sync.dma_start(out=bv_row[:], in_=bv_d[:])

            bv_bcast = wpool.tile([128, C], F32)
            qk = wpool.tile([128, T], BF16)   # rows 0:64 Q, 64:128 K
            kq = wpool.tile([128, T], BF16)   # rows 0:64 K, 64:128 Q
            vT = wpool.tile([128, NSC, C], BF16)

            def emit_proj(tt):
                tsl = slice(tt * TB, (tt + 1) * TB)
                if tt > 0:
                    # issued behind the previous tt's kq swaps on the Sync
                    # queue, so the early fabric bandwidth all goes to the
                    # transfers the projection head is actually waiting on
                    nc.sync.dma_start(out=x_s[:, tt], in_=x_d[:, tt])
                # packed Q/K projection: out rows 0:64 = Q, 64:128 = K
                ps = ps_proj.tile([128, TB], F32, tag="proj", name=f"qkp_{tt}")
                for ci in range(NCH):
                    nc.tensor.matmul(
                        ps[:],
                        wqkT[:, ci, :],
                        x_s[:, tt, ci, :],
                        start=(ci == 0),
                        stop=(ci == NCH - 1),
                    )
                nc.vector.tensor_scalar_add(qk[:, tsl], ps[:], bqk[:, 0:1])
                # swap-duplicate for row-packed score matmuls
                nc.sync.dma_start(out=kq[0:CQ, tsl], in_=qk[CQ:128, tsl])
                nc.sync.dma_start(out=kq[CQ:128, tsl], in_=qk[0:CQ, tsl])

                if tt == 0:
                    # bv broadcast [1, C] -> [128, C] (single bf16 matmul)
                    bvb = ps_proj.tile([128, C], F32, tag="proj", name="bvb")
                    nc.tensor.matmul(
                        bvb[:], ones_row[:], bv_row[:], start=True, stop=True
                    )
                    nc.vector.tensor_copy(bv_bcast[:], bvb[:])

                # V^T projection: vT[s, c] = x.T @ Wv.T + bv
                for jsub in range(NCH):
                    j = 4 * tt + jsub
                    psv = ps_proj.tile([128, C], F32, tag="proj", name=f"vp_{j}")
                    for ci in range(NCH):
                        nc.tensor.matmul(
                            psv[:],
                            x_s[:, tt, ci, jsub * 128:(jsub + 1) * 128],
                            wvT[:, ci, :],
                            start=(ci == 0),
                            stop=(ci == NCH - 1),
                        )
                    nc.vector.tensor_add(vT[:, j, :], psv[:], bv_bcast[:])

            # ---- attention: pair-level software pipeline, one pair deep.
            # Per pair slot the PE does: 2 concurrent score matmuls (~213ns)
            # + 8 accumulating AV matmuls of the previous pair (~1.7us); the
            # exp of the current pair (~1.1us on ACT) hides under that, so
            # sc can be single-buffered and the PE stream stays dense.
            state = {}   # tb -> block state
            avs = {}     # tb -> 4 PSUM accumulators

            def emit_scores_pair(tb, jj, st):
                tsl = slice(tb * TB, (tb + 1) * TB)
                j0, j1 = 2 * jj, 2 * jj + 1
                sc = ps_sc.tile([128, 2, TB], F32, tag="sc", name=f"sc_{tb}_{jj}")
                nc.tensor.matmul(
                    sc[:, 0, :],
                    kq[0:CQ, j0 * 128:(j0 + 1) * 128],
                    qk[0:CQ, tsl],
                    start=True,
                    stop=True,
                )
                nc.tensor.matmul(
                    sc[:, 1, :],
                    qk[CQ:128, j1 * 128:(j1 + 1) * 128],
                    kq[CQ:128, tsl],
                    start=True,
                    stop=True,
                    tile_position=(64, 0),
                )
                etp = et_pool.tile([128, 2, TB], BF16, tag="etp", name=f"etp_{tb}_{jj}")
                nc.scalar.activation(etp[:, :, :], sc[:, :, :], AF.Exp)
                st["etps"].append(etp)

            def emit_tree(tb, jj, st):
                # bf16 pairwise tree-sum toward the softmax denominator (DVE);
                # q-level partials fold straight into the dns PSUM accumulator
                # via ones-matmuls (4 per block), keeping the serial DVE chain
                # at the block boundary short. The reciprocal is kicked off as
                # soon as the last dns matmul lands.
                etp = st["etps"][jj]
                p = tree_pool.tile(
                    [128, TB], BF16, tag="tp", bufs=4, name=f"tp_{tb}_{jj}"
                )
                nc.vector.tensor_add(p[:], etp[:, 0, :], etp[:, 1, :])
                st["p"].append(p)
                if jj % 2 == 1:
                    q = tree_pool.tile(
                        [128, TB], BF16, tag="tq", bufs=3, name=f"tq_{tb}_{jj // 2}"
                    )
                    nc.vector.tensor_add(q[:], st["p"][jj - 1][:], st["p"][jj][:])
                    if jj == 1:
                        st["dns"] = ps_proj.tile(
                            [128, TB], F32, tag="proj", name=f"dns_{tb}"
                        )
                    nc.tensor.matmul(
                        st["dns"][0:1, :],
                        ones_col[:],
                        q[:],
                        start=(jj == 1),
                        stop=(jj == NPAIR - 1),
                    )
                if jj == NPAIR - 1:
                    rcol = small.tile([1, TB], F32, tag="rcol", name=f"rcol_{tb}")
                    nc.vector.reciprocal_approx_fast(rcol[:], st["dns"][0:1, :])
                    rcolb = small.tile([1, TB], BF16, tag="rcolb", name=f"rcolb_{tb}")
                    nc.vector.tensor_copy(rcolb[:], rcol[:])
                    # broadcast 1/denom across partitions on the idle GpSimd
                    rb = rb_pool.tile([128, TB], BF16, tag="rb", name=f"rb_{tb}")
                    nc.gpsimd.partition_broadcast(rb[:], rcolb[:])
                    st["rb"] = rb

            def emit_consume(tb, jj, st):
                if jj == 0:
                    avs[tb] = [
                        ps_av.tile([128, TB], F32, tag=f"av{ck}", name=f"av{ck}_{tb}")
                        for ck in range(NCC)
                    ]
                etp = st["etps"][jj]
                last = jj == NPAIR - 1
                tsl = slice(tb * TB, (tb + 1) * TB)
                # ck-major order so av banks are first touched in the order
                # the previous block's normalize frees them; on the last pair
                # the reciprocal-broadcast matmul slots in after ck1 and the
                # per-ck normalize+store follows each finished accumulator.
                def emit_mul(cko):
                    ot = outp.tile([128, TB], F32, tag="ot", name=f"ot_{tb}_{cko}")
                    nc.vector.tensor_mul(ot[:], avs[tb][cko][:], st["rb"][:])
                    nc.sync.dma_start(
                        out=out_d[cko * 128:(cko + 1) * 128, tsl], in_=ot[:]
                    )

                for ck in range(NCC):
                    for idx in (0, 1):
                        j = 2 * jj + idx
                        nc.tensor.matmul(
                            avs[tb][ck][:],
                            vT[:, j, ck * 128:(ck + 1) * 128],
                            etp[:, idx, :],
                            start=(j == 0),
                            stop=(j == NSC - 1),
                        )
                    if last and ck >= 2:
                        emit_mul(ck - 2)
                if last:
                    emit_mul(2)
                    emit_mul(3)

            for tt in range(NTB):
                emit_proj(tt)

            pending = None  # (tb, jj)
            for tb in range(NTB):
                st = {"etps": [], "p": []}
                state[tb] = st
                for jj in range(NPAIR):
                    emit_scores_pair(tb, jj, st)
                    if pending is not None:
                        ptb, pjj = pending
                        emit_consume(ptb, pjj, state[ptb])
                        if pjj == NPAIR - 1:
                            del state[ptb]
                    emit_tree(tb, jj, st)
                    pending = (tb, jj)
            ptb, pjj = pending
            emit_consume(ptb, pjj, state[ptb])

    nc.compile()
    return nc


_PROGRAM = None


def _get_program() -> bass.Bass:
    global _PROGRAM
    if _PROGRAM is None:
        _PROGRAM = _build_program()
    return _PROGRAM


def _prep_inputs(inputs):
    x = np.ascontiguousarray(np.asarray(inputs["x"], dtype=np.float32))
    wq = np.asarray(inputs["Wq"], dtype=np.float32)
    bq = np.asarray(inputs["bq"], dtype=np.float32).reshape(CQ)
    wk = np.asarray(inputs["Wk"], dtype=np.float32)
    bk = np.asarray(inputs["bk"], dtype=np.float32).reshape(CQ)
    wv = np.asarray(inputs["Wv"], dtype=np.float32)
    bv = np.asarray(inputs["bv"], dtype=np.float32).reshape(C)

    bf = ml_dtypes.bfloat16
    # wqkT[p, ci, m]: m<64 -> Wq[m, ci*128+p], m>=64 -> Wk[m-64, ci*128+p]
    wqk = np.concatenate([wq, wk], axis=0)          # [128, C]
    wqkT = np.ascontiguousarray(
        wqk.T.reshape(NCH, 128, 128).transpose(1, 0, 2)
    ).astype(bf)                                     # [128, NCH, 128]
    # wvT[p, ci, c] = Wv[c, ci*128+p]
    wvT = np.ascontiguousarray(
        wv.T.reshape(NCH, 128, C).transpose(1, 0, 2)
    ).astype(bf)                                     # [128, NCH, C]
    bqk = np.concatenate([bq, bk]).reshape(128, 1).astype(np.float32)
    bv_row = np.ascontiguousarray(bv.reshape(1, C)).astype(bf)
    # x_bf[b][p, tt, ci, t'] = x[b, ci*128+p, tt*TB + t']
    x_bf = np.ascontiguousarray(
        x.reshape(B, NCH, 128, NTB, TB).transpose(0, 2, 3, 1, 4)
    ).astype(bf)                                     # [B, 128, NTB, NCH, TB]

    return [
        {
            "x": np.ascontiguousarray(x_bf[b]),
            "wqkT": wqkT,
            "wvT": wvT,
            "bqk": bqk,
            "bv": bv_row,
        }
        for b in range(NCORES)
    ]


def kernel(**inputs: np.ndarray) -> np.ndarray:
    nc = _get_program()
    in_maps = _prep_inputs(inputs)
    res = run_bass_kernel_spmd(nc, in_maps, list(range(NCORES)))
    out = np.stack([res.results[b]["out"] for b in range(NCORES)], axis=0)
    return out.astype(np.float32)


if __name__ == "__main__":
    import reference

    inputs = {k: np.asarray(v) for k, v in reference.setup_inputs().items()}
    expected = np.asarray(reference.reference(**inputs))
    actual = kernel(**inputs)
    rel = np.linalg.norm(actual - expected) / np.linalg.norm(expected)
    print("Relative error:", rel)
